# revision 18
# baseline (speedup 1.0000x reference)
"""Multi-head attention (B=4, S=2048, D=1024, H=16) on 8 Trainium2 cores.

Sharding: core c -> (batch b=c//2, query-half hq=c%2). Each core computes
K/V projections for its batch's full sequence (no collectives needed) and
attention + output projection for its 1024 query rows.

v3: PE-anchored warm pipeline. The HAM clock gate throttles the PE to
1.2 GHz whenever the matmul stream has recurring micro-gaps, so the
attention loop is balanced such that warm-PE work per iteration slightly
EXCEEDS the ScalarE exp cadence (~1.11us per [128,1024] exp):
  - per iteration (one head, 2 k-tiles): 2 score matmuls + 2 ctx matmuls
    (2048 cy) plus deferred filler matmuls popped from a queue.
  - qc=0 filler: k-projection for the next head-pair's feature tile
    (~1024 cy/iter).  qc=1 filler: q-projection (qc1 half) of the next
    feature tile + out-proj(qc0) groups (~512 cy/iter).
  - per-head softmax normalization: the 3.3us DVE reciprocal is issued 4
    iterations before the PE broadcast matmul that consumes it.
This is self-correcting: if the PE ever drops to 1.2 GHz it becomes
strictly the bottleneck, runs back-to-back, and re-warms.

Device dataflow (all activations kept transposed, [feature, seq]):
  qT[e,s]   = WqT.T-contract  (lhsT=WqT[d,e] tiles, rhs=xT[d,s])
  kT[e,s]   = same with WkT
  v[s,e]    = lhsT=xT[d,s] tiles, rhs=WvT[d,e]  (+bias via K=1 ones matmul)
  scoresT[k,q] = kT_h.T-contract q; exp via ScalarE (scale=0.125) -> bf16
  ctxT[dv,q]  += [v_h | ones] @ expT   (row 64 = softmax denominator)
  normalize: reciprocal + PE broadcast outer-product + DVE multiply
  outT[e,q] = WoT.T-contract ctxnT  (bias bo added host-side)
Host: out[b, hq*1024:(hq+1)*1024, :] = outT.T + bo
"""

import numpy as np
import ml_dtypes

import concourse.bacc as bacc
import concourse.tile as tile
from concourse import mybir
from concourse.bass_utils import run_bass_kernel_spmd

B, S, D = 4, 2048, 1024
H, HD = 16, 64
SQ = 1024          # query rows per core
NDT = D // 128     # 8 d-tiles
NET = D // 128     # 8 e-tiles
NKT = S // 128     # 16 k-tiles
NST = S // 128     # 16 s-tiles
NQC = SQ // 512    # 2 q-chunks per core
BF16 = mybir.dt.bfloat16
F32 = mybir.dt.float32
SCALE = 1.0 / 8.0  # 1/sqrt(HD)

_NC_CACHE = None


def build_nc():
    nc = bacc.Bacc(None, target_bir_lowering=False, debug=True)

    xT_d = nc.declare_dram_parameter("xT", [D, S], BF16, isOutput=False)
    WqT_d = nc.declare_dram_parameter("WqT", [D, D], BF16, isOutput=False)
    WkT_d = nc.declare_dram_parameter("WkT", [D, D], BF16, isOutput=False)
    WvT_d = nc.declare_dram_parameter("WvT", [D, D], BF16, isOutput=False)
    WoT_d = nc.declare_dram_parameter("WoT", [D, D], BF16, isOutput=False)
    bqt_d = nc.declare_dram_parameter("bqt", [128, NET], F32, isOutput=False)
    bkt_d = nc.declare_dram_parameter("bkt", [128, NET], F32, isOutput=False)
    bvr_d = nc.declare_dram_parameter("bvr", [1, D], BF16, isOutput=False)
    outT_d = nc.declare_dram_parameter("outT", [D, SQ], F32, isOutput=True)

    with tile.TileContext(nc) as tc:
        with tc.tile_pool(name="resident", bufs=1) as res:
            # ---- resident SBUF tensors ----
            kT = [res.tile([128, S], BF16, name=f"kT{t}", tag=f"kT{t}")
                  for t in range(NET)]
            # separate q tiles per q-chunk so the deferred qc1 projection
            # never write-aliases the tile qc0 scores are reading
            qTa = [res.tile([128, 512], BF16, name=f"qTa{t}", tag=f"qTa{t}")
                   for t in range(NET)]
            qTb = [res.tile([128, 512], BF16, name=f"qTb{t}", tag=f"qTb{t}")
                   for t in range(NET)]
            vv = [res.tile([128, H, HD + 1], BF16, name=f"v{t}", tag=f"v{t}")
                  for t in range(NST)]
            ctxn = [res.tile([128, SQ], BF16, name=f"ctxn{t}", tag=f"ctxn{t}")
                    for t in range(NDT)]
            # xT, Wk, Wq stay resident: k-proj / q-proj(qc1) are deferred
            # into the attention loops as PE filler.  The wko tiles hold
            # Wk through qc0, then Wo is DMA'd over them for qc1 (the WAR
            # dep on the last k-proj matmul orders the overwrite).
            xT = [res.tile([128, S], BF16, name=f"xT{t}", tag=f"xT{t}")
                  for t in range(NDT)]
            wko_t = [res.tile([128, D], BF16, name=f"wko{t}", tag=f"wko{t}")
                     for t in range(NDT)]
            wq_t = [res.tile([128, D], BF16, name=f"wq{t}", tag=f"wq{t}")
                    for t in range(NDT)]
            bq_dma = res.tile([128, NET], F32, tag="bq_dma")
            bk_dma = res.tile([128, NET], F32, tag="bk_dma")
            bq_sb = res.tile([128, NET], F32, tag="bq_sb")
            bk_sb = res.tile([128, NET], F32, tag="bk_sb")
            bv_sb = res.tile([1, D], BF16, tag="bv_sb")
            ones_bf = res.tile([1, 128], BF16, tag="ones_bf")
            ones_r = res.tile([65, 64], F32, tag="ones_r")

            nc.sync.dma_start(out=bq_dma, in_=bqt_d[:, :])
            nc.sync.dma_start(out=bk_dma, in_=bkt_d[:, :])
            nc.sync.dma_start(out=bv_sb, in_=bvr_d[:, :])
            # TensorScalarPtr has a single sync-wait slot; route the biases
            # through DVE once so later readers rely on program order.
            nc.vector.tensor_copy(out=bq_sb, in_=bq_dma)
            nc.vector.tensor_copy(out=bk_sb, in_=bk_dma)
            nc.vector.memset(ones_bf, 1.0)
            nc.vector.memset(ones_r, 1.0)
            warmup = res.tile([1, 16], F32, tag="warmup")
            nc.scalar.activation(warmup, ones_r[0:1, 0:16],
                                 mybir.ActivationFunctionType.Exp, scale=1.0)
            for t in range(NST):
                # only the denominator column; cols 0:HD are overwritten
                nc.vector.memset(vv[t][:, :, HD:HD + 1], 1.0)



            def qproj_thunks(et, qtile, q0):
                kp_ps = kop.tile([128, 512], F32, name="kp", tag="kop",
                                 bufs=1)

                def mm(dt, et=et, kp=kp_ps, qtile=qtile, q0=q0):
                    nc.tensor.matmul(
                        kp,
                        wq_t[dt][:, et * 128:(et + 1) * 128],
                        xT[dt][:, q0:q0 + 512],
                        start=(dt == 0), stop=(dt == NDT - 1))
                    if dt == NDT - 1:
                        # evac with the last matmul: lands in the DVE queue
                        # ahead of later-emitted normalization ops
                        nc.vector.tensor_scalar_add(
                            out=qtile, in0=kp, scalar1=bq_sb[:, et:et + 1])
                return [lambda dt=dt, mm=mm: mm(dt) for dt in range(NDT)]

            def kproj_thunks(et):
                th = []
                for sc in range(S // 512):
                    kp_ps = kop.tile([128, 512], F32, name="kp", tag="kop",
                                     bufs=1)

                    def mm(dt, et=et, sc=sc, kp=kp_ps):
                        nc.tensor.matmul(
                            kp,
                            wko_t[dt][:, et * 128:(et + 1) * 128],
                            xT[dt][:, sc * 512: sc * 512 + 512],
                            start=(dt == 0), stop=(dt == NDT - 1))
                        if dt == NDT - 1:
                            nc.vector.tensor_scalar_add(
                                out=kT[et][:, sc * 512:(sc + 1) * 512],
                                in0=kp,
                                scalar1=bk_sb[:, et:et + 1])
                    th.extend(lambda dt=dt, mm=mm: mm(dt)
                              for dt in range(NDT))
                return th

            # ================= phase 1 =================
            # v-proj first (warms the PE while later weights stream in),
            # then q-proj (qc0 half all tiles, qc1 half et=0).
            # k-proj (incl. et=0) and qTb[1..7] are deferred into the
            # attention loops.
            with tc.tile_pool(name="p1", bufs=1) as p1, \
                 tc.psum_pool(name="pp", bufs=4) as pp:
                wv_t = []
                for t in range(NDT):
                    wt = p1.tile([128, D], BF16, name=f"wv{t}", tag="wrot",
                                 bufs=8)
                    nc.sync.dma_start(out=wt, in_=WvT_d[t * 128:(t + 1) * 128, :])
                    nc.sync.dma_start(out=xT[t][:, 0:S // 2],
                                      in_=xT_d[t * 128:(t + 1) * 128, 0:S // 2])
                    wv_t.append(wt)
                for t in range(NDT):
                    nc.sync.dma_start(out=xT[t][:, S // 2:S],
                                      in_=xT_d[t * 128:(t + 1) * 128, S // 2:S])
                for t in range(NDT):
                    nc.sync.dma_start(out=wq_t[t],
                                      in_=WqT_d[t * 128:(t + 1) * 128, :])

                # v: out[s_tile, e_chunk] accumulated over d, + ones-row bias
                for st in range(NST):
                    for ec in range(D // 512):
                        ps = pp.tile([128, 512], F32, name="ps", tag="proj")
                        for dt in range(NDT):
                            nc.tensor.matmul(
                                ps,
                                xT[dt][:, st * 128:(st + 1) * 128],
                                wv_t[dt][:, ec * 512:(ec + 1) * 512],
                                start=(dt == 0), stop=False)
                        nc.tensor.matmul(
                            ps,
                            ones_bf[0:1, 0:128],
                            bv_sb[0:1, ec * 512:(ec + 1) * 512],
                            start=False, stop=True)
                        nc.vector.tensor_copy(
                            out=vv[st][:, ec * 8:(ec + 1) * 8, 0:HD],
                            in_=ps.rearrange("p (h d) -> p h d", h=8))

                # late DMA: wk needed from qc0 start
                for t in range(NDT):
                    nc.sync.dma_start(out=wko_t[t],
                                      in_=WkT_d[t * 128:(t + 1) * 128, :])

                # q (qc0 half, all tiles)
                for et in range(NET):
                    ps = pp.tile([128, 512], F32, name="ps", tag="proj")
                    for dt in range(NDT):
                        nc.tensor.matmul(
                            ps,
                            wq_t[dt][:, et * 128:(et + 1) * 128],
                            xT[dt][:, 0:512],
                            start=(dt == 0), stop=(dt == NDT - 1))
                    nc.vector.tensor_scalar_add(
                        out=qTa[et], in0=ps, scalar1=bq_sb[:, et:et + 1])

                # qTb[0]
                ps = pp.tile([128, 512], F32, name="ps", tag="proj")
                for dt in range(NDT):
                    nc.tensor.matmul(
                        ps, wq_t[dt][:, 0:128], xT[dt][:, 512:1024],
                        start=(dt == 0), stop=(dt == NDT - 1))
                nc.vector.tensor_scalar_add(
                    out=qTb[0], in0=ps, scalar1=bq_sb[:, 0:1])

            # ================= phase 2: attention + out-proj =================
            with tc.tile_pool(name="p2", bufs=1) as p2, \
                 tc.psum_pool(name="sp", bufs=2) as sp, \
                 tc.psum_pool(name="cp", bufs=2) as cp, \
                 tc.psum_pool(name="kop", bufs=2) as kop:

                inv_of = {}       # key -> inv tile (sbuf [1,512])

                def emit_recip(ctx_ps, key):
                    iv = p2.tile([1, 512], F32, name="inv", tag="inv", bufs=3)
                    nc.vector.reciprocal(iv, ctx_ps[64:65, :])
                    inv_of[key] = iv

                def emit_bc(key):
                    # broadcast inv across partitions on the (idle) GpSimd
                    bc_sb = p2.tile([64, 512], F32, name="bc_sb",
                                    tag="bc_sb", bufs=2)
                    nc.gpsimd.partition_broadcast(bc_sb, inv_of.pop(key))
                    inv_of[key] = bc_sb

                def emit_bcmul(ctx_ps, key, h, qc):
                    ht, hp = h // 2, (h % 2) * 64
                    nc.vector.tensor_mul(
                        ctxn[ht][hp:hp + 64, qc * 512:(qc + 1) * 512],
                        ctx_ps[0:64, :], inv_of.pop(key))

                oq = []           # out-proj thunk queue (qc1 filler)

                def emit_out_group(qc_o, et):
                    ps = kop.tile([128, 512], F32, name="ops", tag="kop",
                                  bufs=1)
                    for dt in range(NDT):
                        oq.append(lambda dt=dt, et=et, qc_o=qc_o, ps=ps:
                                  nc.tensor.matmul(
                                      ps,
                                      wko_t[dt][:, et * 128:(et + 1) * 128],
                                      ctxn[dt][:, qc_o * 512:(qc_o + 1) * 512],
                                      start=(dt == 0), stop=(dt == NDT - 1)))

                    def evac(ps=ps, et=et, qc_o=qc_o):
                        osb = p2.tile([128, 512], F32, name="osb", tag="osb",
                                      bufs=2)
                        nc.vector.tensor_copy(out=osb, in_=ps)
                        nc.gpsimd.dma_start(
                            out=outT_d[et * 128:(et + 1) * 128,
                                       qc_o * 512:(qc_o + 1) * 512],
                            in_=osb)
                    oq.append(evac)

                pend = None       # (ctx_ps, head, qc) awaiting normalization

                def attention_head(qc, h, fq, pops):
                    """One head's attention: 8 iters x 2 k-tiles.
                    fq: filler thunk queue, `pops` popped per iteration."""
                    nonlocal pend
                    ht, hp = h // 2, (h % 2) * 64
                    qtile = qTa[ht] if qc == 0 else qTb[ht]
                    ctx_ps = cp.tile([65, 512], F32, name="ctx", tag="ctx",
                                     bufs=2)
                    expq = []
                    for kh in range(NKT // 2):
                        gidx = h * 8 + kh
                        while fq and fq[0][0] <= gidx:
                            fq.pop(0)[1]()
                        sc_ps = sp.tile([128, 1024], F32, name="sc_ps",
                                        tag="sc", bufs=2)
                        for j in range(2):
                            kt = 2 * kh + j
                            nc.tensor.matmul(
                                sc_ps[:, j * 512:(j + 1) * 512],
                                kT[ht][hp:hp + 64, kt * 128:(kt + 1) * 128],
                                qtile[hp:hp + 64, :],
                                start=True, stop=True)
                        expT = p2.tile([128, 1024], BF16, name="expT",
                                       tag="expT", bufs=3)
                        nc.scalar.activation(
                            expT, sc_ps,
                            mybir.ActivationFunctionType.Exp,
                            scale=SCALE)
                        expq.append((expT, kh))
                        n_pop = pops + (1 if kh < 4 else 0)
                        for _ in range(n_pop):
                            if fq:
                                fq.pop(0)[1]()
                        if pend is not None:
                            pctx, ph, pqc = pend
                            if kh == 1:
                                emit_recip(pctx, (pqc, ph))
                            elif kh == 4:
                                emit_bc((pqc, ph))
                            elif kh == 5:
                                emit_bcmul(pctx, (pqc, ph), ph, pqc)
                        if kh >= 1:
                            eT, ekh = expq.pop(0)
                            for j, kt in ((0, 2 * ekh), (1, 2 * ekh + 1)):
                                nc.tensor.matmul(
                                    ctx_ps, vv[kt][:, h, :],
                                    eT[:, j * 512:(j + 1) * 512],
                                    start=(kt == 0), stop=(kt == NKT - 1))
                    eT, ekh = expq.pop(0)
                    for j, kt in ((0, 2 * ekh), (1, 2 * ekh + 1)):
                        nc.tensor.matmul(
                            ctx_ps, vv[kt][:, h, :],
                            eT[:, j * 512:(j + 1) * 512],
                            start=False, stop=(kt == NKT - 1))
                    pend = (ctx_ps, h, qc)

                # ---------------- qc = 0 ----------------
                fq = []
                for h in range(H):
                    ets = [0, 1] if h == 0 else (
                        [h // 2 + 1] if (h % 2 == 0 and h // 2 + 1 < NET)
                        else [])
                    for et in ets:
                        th = kproj_thunks(et)
                        # thunk i belongs to chunk i//9; chunk sc must land
                        # before head 2*et reads kT cols at iter kh=2*sc
                        fq.extend((16 * et + 2 * (i // 8), t)
                                  for i, t in enumerate(th))
                    attention_head(0, h, fq, 2)
                while fq:
                    fq.pop(0)[1]()
                for t in range(NDT):
                    nc.sync.dma_start(out=wko_t[t],
                                      in_=WoT_d[t * 128:(t + 1) * 128, :])

                # ---------------- qc = 1 ----------------
                for h in range(H):
                    if h % 2 == 0 and h // 2 + 1 < NET:
                        et = h // 2 + 1
                        fq.extend((16 * et, th)
                                  for th in qproj_thunks(et, qTb[et], 512))
                    if h >= 2 and h % 2 == 0:
                        emit_out_group(0, h // 2 - 1)
                        if h == H - 2:
                            emit_out_group(0, NET - 1)
                        fq.extend((9999, th) for th in oq)
                        oq.clear()
                    attention_head(1, h, fq, 1)
                while fq:
                    fq.pop(0)[1]()

                # tail: last head's norm, remaining out-proj groups
                pctx, ph, pqc = pend
                emit_recip(pctx, (pqc, ph))
                emit_bc((pqc, ph))
                emit_bcmul(pctx, (pqc, ph), ph, pqc)
                for et in range(NET):
                    emit_out_group(1, et)
                while oq:
                    oq.pop(0)()
    nc.compile()
    return nc


def _get_nc():
    global _NC_CACHE
    if _NC_CACHE is None:
        _NC_CACHE = build_nc()
    return _NC_CACHE


def _prep_maps(x, Wq, bq, Wk, bk, Wv, bv, Wo):
    bf = ml_dtypes.bfloat16
    WqT = np.ascontiguousarray(Wq.T).astype(bf)
    WkT = np.ascontiguousarray(Wk.T).astype(bf)
    WvT = np.ascontiguousarray(Wv.T).astype(bf)
    WoT = np.ascontiguousarray(Wo.T).astype(bf)
    bqt = np.ascontiguousarray(bq.reshape(NET, 128).T).astype(np.float32)
    bkt = np.ascontiguousarray(bk.reshape(NET, 128).T).astype(np.float32)
    bvr = np.ascontiguousarray(bv.reshape(1, D)).astype(bf)
    in_maps = []
    for c in range(8):
        b, hq = c // 2, c % 2
        xTb = np.ascontiguousarray(x[b].T).astype(bf)  # [D, S]
        if hq == 1:
            # rotate so local query half sits at columns [0, SQ)
            xTb = np.ascontiguousarray(
                np.concatenate([xTb[:, SQ:], xTb[:, :SQ]], axis=1))
        in_maps.append(dict(xT=xTb, WqT=WqT, WkT=WkT, WvT=WvT, WoT=WoT,
                            bqt=bqt, bkt=bkt, bvr=bvr))
    return in_maps


def run(x, Wq, bq, Wk, bk, Wv, bv, Wo, bo, trace=False, **spmd_kwargs):
    nc = _get_nc()
    in_maps = _prep_maps(x, Wq, bq, Wk, bk, Wv, bv, Wo)
    res = run_bass_kernel_spmd(nc, in_maps, core_ids=list(range(8)),
                               trace=trace, **spmd_kwargs)
    out = np.empty((B, S, D), np.float32)
    for c in range(8):
        b, hq = c // 2, c % 2
        out[b, hq * SQ:(hq + 1) * SQ, :] = np.asarray(
            res.results[c]["outT"], np.float32).T
    out += bo.astype(np.float32)
    return out, res


def kernel(x, Wq, bq, Wk, bk, Wv, bv, Wo, bo):
    out, _ = run(np.asarray(x, np.float32), np.asarray(Wq, np.float32),
                 np.asarray(bq, np.float32), np.asarray(Wk, np.float32),
                 np.asarray(bk, np.float32), np.asarray(Wv, np.float32),
                 np.asarray(bv, np.float32), np.asarray(Wo, np.float32),
                 np.asarray(bo, np.float32))
    return out


# revision 19
# speedup vs baseline: 1.1093x; 1.1093x over previous
"""Multi-head attention (B=4, S=2048, D=1024, H=16) on 8 Trainium2 cores.

Sharding: core c -> (batch b=c//2, query-half hq=c%2). Each core computes
K/V projections for its batch's full sequence (no collectives needed) and
attention + output projection for its 1024 query rows.

v3: PE-anchored warm pipeline. The HAM clock gate throttles the PE to
1.2 GHz whenever the matmul stream has recurring micro-gaps, so the
attention loop is balanced such that warm-PE work per iteration slightly
EXCEEDS the ScalarE exp cadence (~1.11us per [128,1024] exp):
  - per iteration (one head, 2 k-tiles): 2 score matmuls + 2 ctx matmuls
    (2048 cy) plus deferred filler matmuls popped from a queue.
  - qc=0 filler: k-projection for the next head-pair's feature tile
    (~1024 cy/iter).  qc=1 filler: q-projection (qc1 half) of the next
    feature tile + out-proj(qc0) groups (~512 cy/iter).
  - per-head softmax normalization: the 3.3us DVE reciprocal is issued 4
    iterations before the PE broadcast matmul that consumes it.
This is self-correcting: if the PE ever drops to 1.2 GHz it becomes
strictly the bottleneck, runs back-to-back, and re-warms.

Device dataflow (all activations kept transposed, [feature, seq]):
  qT[e,s]   = WqT.T-contract  (lhsT=WqT[d,e] tiles, rhs=xT[d,s])
  kT[e,s]   = same with WkT
  v[s,e]    = lhsT=xT[d,s] tiles, rhs=WvT[d,e]  (+bias via K=1 ones matmul)
  scoresT[k,q] = kT_h.T-contract q; exp via ScalarE (scale=0.125) -> bf16
  ctxT[dv,q]  += [v_h | ones] @ expT   (row 64 = softmax denominator)
  normalize: reciprocal + PE broadcast outer-product + DVE multiply
  outT[e,q] = WoT.T-contract ctxnT  (bias bo added host-side)
Host: out[b, hq*1024:(hq+1)*1024, :] = outT.T + bo
"""

import numpy as np
import ml_dtypes

import concourse.bacc as bacc
import concourse.tile as tile
from concourse import mybir
from concourse.bass_utils import run_bass_kernel_spmd

B, S, D = 4, 2048, 1024
H, HD = 16, 64
SQ = 1024          # query rows per core
NDT = D // 128     # 8 d-tiles
NET = D // 128     # 8 e-tiles
NKT = S // 128     # 16 k-tiles
NST = S // 128     # 16 s-tiles
NQC = SQ // 512    # 2 q-chunks per core
BF16 = mybir.dt.bfloat16
F32 = mybir.dt.float32
SCALE = 1.0 / 8.0  # 1/sqrt(HD)

_NC_CACHE = None


def build_nc():
    nc = bacc.Bacc(None, target_bir_lowering=False, debug=True)

    xT_d = nc.declare_dram_parameter("xT", [D, S], BF16, isOutput=False)
    WqT_d = nc.declare_dram_parameter("WqT", [D, D], BF16, isOutput=False)
    WkT_d = nc.declare_dram_parameter("WkT", [D, D], BF16, isOutput=False)
    WvT_d = nc.declare_dram_parameter("WvT", [D, D], BF16, isOutput=False)
    WoT_d = nc.declare_dram_parameter("WoT", [D, D], BF16, isOutput=False)
    bqt_d = nc.declare_dram_parameter("bqt", [128, NET], F32, isOutput=False)
    bkt_d = nc.declare_dram_parameter("bkt", [128, NET], F32, isOutput=False)
    bvr_d = nc.declare_dram_parameter("bvr", [1, D], BF16, isOutput=False)
    outT_d = nc.declare_dram_parameter("outT", [D, SQ], F32, isOutput=True)

    with tile.TileContext(nc) as tc:
        with tc.tile_pool(name="resident", bufs=1) as res:
            # ---- resident SBUF tensors ----
            kT = [res.tile([128, S], BF16, name=f"kT{t}", tag=f"kT{t}")
                  for t in range(NET)]
            # separate q tiles per q-chunk so the deferred qc1 projection
            # never write-aliases the tile qc0 scores are reading
            qTa = [res.tile([128, 512], BF16, name=f"qTa{t}", tag=f"qTa{t}")
                   for t in range(NET)]
            qTb = [res.tile([128, 512], BF16, name=f"qTb{t}", tag=f"qTb{t}")
                   for t in range(NET)]
            vv = [res.tile([128, H, HD + 1], BF16, name=f"v{t}", tag=f"v{t}")
                  for t in range(NST)]
            ctxn = [res.tile([128, SQ], BF16, name=f"ctxn{t}", tag=f"ctxn{t}")
                    for t in range(NDT)]
            # xT, Wk, Wq stay resident: k-proj / q-proj(qc1) are deferred
            # into the attention loops as PE filler.  The wko tiles hold
            # Wk through qc0, then Wo is DMA'd over them for qc1 (the WAR
            # dep on the last k-proj matmul orders the overwrite).
            xT = [res.tile([128, S], BF16, name=f"xT{t}", tag=f"xT{t}")
                  for t in range(NDT)]
            wko_t = [res.tile([128, D], BF16, name=f"wko{t}", tag=f"wko{t}")
                     for t in range(NDT)]
            wq_t = [res.tile([128, D], BF16, name=f"wq{t}", tag=f"wq{t}")
                    for t in range(NDT)]
            bq_dma = res.tile([128, NET], F32, tag="bq_dma")
            bk_dma = res.tile([128, NET], F32, tag="bk_dma")
            bq_sb = res.tile([128, NET], F32, tag="bq_sb")
            bk_sb = res.tile([128, NET], F32, tag="bk_sb")
            bv_sb = res.tile([1, D], BF16, tag="bv_sb")
            ones_bf = res.tile([1, 128], BF16, tag="ones_bf")
            ones_r = res.tile([65, 64], F32, tag="ones_r")

            nc.sync.dma_start(out=bq_dma, in_=bqt_d[:, :])
            nc.sync.dma_start(out=bk_dma, in_=bkt_d[:, :])
            nc.sync.dma_start(out=bv_sb, in_=bvr_d[:, :])
            # TensorScalarPtr has a single sync-wait slot; route the biases
            # through DVE once so later readers rely on program order.
            nc.vector.tensor_copy(out=bq_sb, in_=bq_dma)
            nc.vector.tensor_copy(out=bk_sb, in_=bk_dma)
            nc.vector.memset(ones_bf, 1.0)
            nc.vector.memset(ones_r, 1.0)
            warmup = res.tile([1, 16], F32, tag="warmup")
            nc.scalar.activation(warmup, ones_r[0:1, 0:16],
                                 mybir.ActivationFunctionType.Exp, scale=1.0)
            for t in range(NST):
                # only the denominator column; cols 0:HD are overwritten
                nc.vector.memset(vv[t][:, :, HD:HD + 1], 1.0)



            def qproj_thunks(et, qtile, q0):
                th = []
                kp_ps = kop.tile([128, 512], F32, name="kp", tag="kop",
                                 bufs=1)
                for dt in range(NDT):
                    th.append(lambda dt=dt, et=et, kp=kp_ps:
                              nc.tensor.matmul(
                                  kp,
                                  wq_t[dt][:, et * 128:(et + 1) * 128],
                                  xT[dt][:, q0:q0 + 512],
                                  start=(dt == 0), stop=(dt == NDT - 1)))

                def evac(et=et, kp=kp_ps, qtile=qtile):
                    nc.vector.tensor_scalar_add(
                        out=qtile, in0=kp, scalar1=bq_sb[:, et:et + 1])
                th.append(evac)
                return th

            def kproj_thunks(et):
                th = []
                for sc in range(S // 512):
                    kp_ps = kop.tile([128, 512], F32, name="kp", tag="kop",
                                     bufs=1)
                    for dt in range(NDT):
                        th.append(lambda dt=dt, et=et, sc=sc, kp=kp_ps:
                                  nc.tensor.matmul(
                                      kp,
                                      wko_t[dt][:, et * 128:(et + 1) * 128],
                                      xT[dt][:, sc * 512: sc * 512 + 512],
                                      start=(dt == 0), stop=(dt == NDT - 1)))

                    def evac(et=et, sc=sc, kp=kp_ps):
                        nc.vector.tensor_scalar_add(
                            out=kT[et][:, sc * 512:(sc + 1) * 512],
                            in0=kp,
                            scalar1=bk_sb[:, et:et + 1])
                    th.append(evac)
                return th

            # ================= phase 1 =================
            # v-proj first (warms the PE while later weights stream in),
            # then q-proj (qc0 half all tiles, qc1 half et=0).
            # k-proj (incl. et=0) and qTb[1..7] are deferred into the
            # attention loops.
            with tc.tile_pool(name="p1", bufs=1) as p1, \
                 tc.psum_pool(name="pp", bufs=4) as pp:
                wv_t = []
                for t in range(NDT):
                    wt = p1.tile([128, D], BF16, name=f"wv{t}", tag="wrot",
                                 bufs=8)
                    nc.sync.dma_start(out=wt, in_=WvT_d[t * 128:(t + 1) * 128, :])
                    nc.sync.dma_start(out=xT[t][:, 0:S // 2],
                                      in_=xT_d[t * 128:(t + 1) * 128, 0:S // 2])
                    wv_t.append(wt)
                for t in range(NDT):
                    nc.sync.dma_start(out=xT[t][:, S // 2:S],
                                      in_=xT_d[t * 128:(t + 1) * 128, S // 2:S])
                for t in range(NDT):
                    nc.sync.dma_start(out=wq_t[t],
                                      in_=WqT_d[t * 128:(t + 1) * 128, :])

                # v: out[s_tile, e_chunk] accumulated over d, + ones-row bias
                for st in range(NST):
                    for ec in range(D // 512):
                        ps = pp.tile([128, 512], F32, name="ps", tag="proj")
                        for dt in range(NDT):
                            nc.tensor.matmul(
                                ps,
                                xT[dt][:, st * 128:(st + 1) * 128],
                                wv_t[dt][:, ec * 512:(ec + 1) * 512],
                                start=(dt == 0), stop=False)
                        nc.tensor.matmul(
                            ps,
                            ones_bf[0:1, 0:128],
                            bv_sb[0:1, ec * 512:(ec + 1) * 512],
                            start=False, stop=True)
                        nc.vector.tensor_copy(
                            out=vv[st][:, ec * 8:(ec + 1) * 8, 0:HD],
                            in_=ps.rearrange("p (h d) -> p h d", h=8))

                # late DMA: wk needed from qc0 start
                for t in range(NDT):
                    nc.sync.dma_start(out=wko_t[t],
                                      in_=WkT_d[t * 128:(t + 1) * 128, :])

                # q (qc0 half, all tiles)
                for et in range(NET):
                    ps = pp.tile([128, 512], F32, name="ps", tag="proj")
                    for dt in range(NDT):
                        nc.tensor.matmul(
                            ps,
                            wq_t[dt][:, et * 128:(et + 1) * 128],
                            xT[dt][:, 0:512],
                            start=(dt == 0), stop=(dt == NDT - 1))
                    nc.vector.tensor_scalar_add(
                        out=qTa[et], in0=ps, scalar1=bq_sb[:, et:et + 1])

                # qTb[0]
                ps = pp.tile([128, 512], F32, name="ps", tag="proj")
                for dt in range(NDT):
                    nc.tensor.matmul(
                        ps, wq_t[dt][:, 0:128], xT[dt][:, 512:1024],
                        start=(dt == 0), stop=(dt == NDT - 1))
                nc.vector.tensor_scalar_add(
                    out=qTb[0], in0=ps, scalar1=bq_sb[:, 0:1])

            # ================= phase 2: attention + out-proj =================
            with tc.tile_pool(name="p2", bufs=1) as p2, \
                 tc.psum_pool(name="sp", bufs=2) as sp, \
                 tc.psum_pool(name="cp", bufs=2) as cp, \
                 tc.psum_pool(name="kop", bufs=2) as kop:

                inv_of = {}       # key -> inv tile (sbuf [1,512])

                def emit_recip(ctx_ps, key):
                    iv = p2.tile([1, 512], F32, name="inv", tag="inv", bufs=3)
                    nc.vector.reciprocal(iv, ctx_ps[64:65, :])
                    inv_of[key] = iv

                def emit_bc(key):
                    # broadcast inv across partitions on the (idle) GpSimd
                    bc_sb = p2.tile([64, 512], F32, name="bc_sb",
                                    tag="bc_sb", bufs=2)
                    nc.gpsimd.partition_broadcast(bc_sb, inv_of.pop(key))
                    inv_of[key] = bc_sb

                def emit_bcmul(ctx_ps, key, h, qc):
                    ht, hp = h // 2, (h % 2) * 64
                    nc.vector.tensor_mul(
                        ctxn[ht][hp:hp + 64, qc * 512:(qc + 1) * 512],
                        ctx_ps[0:64, :], inv_of.pop(key))

                oq = []           # out-proj thunk queue (qc1 filler)

                def emit_out_group(qc_o, et):
                    ps = kop.tile([128, 512], F32, name="ops", tag="kop",
                                  bufs=1)
                    for dt in range(NDT):
                        oq.append(lambda dt=dt, et=et, qc_o=qc_o, ps=ps:
                                  nc.tensor.matmul(
                                      ps,
                                      wko_t[dt][:, et * 128:(et + 1) * 128],
                                      ctxn[dt][:, qc_o * 512:(qc_o + 1) * 512],
                                      start=(dt == 0), stop=(dt == NDT - 1)))

                    def evac(ps=ps, et=et, qc_o=qc_o):
                        osb = p2.tile([128, 512], F32, name="osb", tag="osb",
                                      bufs=2)
                        nc.vector.tensor_copy(out=osb, in_=ps)
                        nc.gpsimd.dma_start(
                            out=outT_d[et * 128:(et + 1) * 128,
                                       qc_o * 512:(qc_o + 1) * 512],
                            in_=osb)
                    oq.append(evac)

                pend = None       # (ctx_ps, head, qc) awaiting normalization

                def attention_head(qc, h, fq, pops):
                    """One head's attention: 8 iters x 2 k-tiles.
                    fq: filler thunk queue, `pops` popped per iteration."""
                    nonlocal pend
                    ht, hp = h // 2, (h % 2) * 64
                    qtile = qTa[ht] if qc == 0 else qTb[ht]
                    ctx_ps = cp.tile([65, 512], F32, name="ctx", tag="ctx",
                                     bufs=2)
                    expq = []
                    for kh in range(NKT // 2):
                        gidx = h * 8 + kh
                        while fq and fq[0][0] <= gidx:
                            fq.pop(0)[1]()
                        sc_ps = sp.tile([128, 1024], F32, name="sc_ps",
                                        tag="sc", bufs=2)
                        for j in range(2):
                            kt = 2 * kh + j
                            nc.tensor.matmul(
                                sc_ps[:, j * 512:(j + 1) * 512],
                                kT[ht][hp:hp + 64, kt * 128:(kt + 1) * 128],
                                qtile[hp:hp + 64, :],
                                start=True, stop=True)
                        expT = p2.tile([128, 1024], BF16, name="expT",
                                       tag="expT", bufs=3)
                        nc.scalar.activation(
                            expT, sc_ps,
                            mybir.ActivationFunctionType.Exp,
                            scale=SCALE)
                        expq.append((expT, kh))
                        n_pop = pops + (1 if kh < 4 else 0)
                        for _ in range(n_pop):
                            if fq:
                                fq.pop(0)[1]()
                        if pend is not None:
                            pctx, ph, pqc = pend
                            if kh == 1:
                                emit_recip(pctx, (pqc, ph))
                            elif kh == 4:
                                emit_bc((pqc, ph))
                            elif kh == 5:
                                emit_bcmul(pctx, (pqc, ph), ph, pqc)
                        if kh >= 1:
                            eT, ekh = expq.pop(0)
                            for j, kt in ((0, 2 * ekh), (1, 2 * ekh + 1)):
                                nc.tensor.matmul(
                                    ctx_ps, vv[kt][:, h, :],
                                    eT[:, j * 512:(j + 1) * 512],
                                    start=(kt == 0), stop=(kt == NKT - 1))
                    eT, ekh = expq.pop(0)
                    for j, kt in ((0, 2 * ekh), (1, 2 * ekh + 1)):
                        nc.tensor.matmul(
                            ctx_ps, vv[kt][:, h, :],
                            eT[:, j * 512:(j + 1) * 512],
                            start=False, stop=(kt == NKT - 1))
                    pend = (ctx_ps, h, qc)

                # ---------------- qc = 0 ----------------
                fq = []
                for h in range(H):
                    ets = [0, 1] if h == 0 else (
                        [h // 2 + 1] if (h % 2 == 0 and h // 2 + 1 < NET)
                        else [])
                    for et in ets:
                        th = kproj_thunks(et)
                        # thunk i belongs to chunk i//9; chunk sc must land
                        # before head 2*et reads kT cols at iter kh=2*sc
                        fq.extend((16 * et + 2 * (i // 9), t)
                                  for i, t in enumerate(th))
                    attention_head(0, h, fq, 2)
                while fq:
                    fq.pop(0)[1]()
                for t in range(NDT):
                    nc.sync.dma_start(out=wko_t[t],
                                      in_=WoT_d[t * 128:(t + 1) * 128, :])

                # ---------------- qc = 1 ----------------
                for h in range(H):
                    if h % 2 == 0 and h // 2 + 1 < NET:
                        et = h // 2 + 1
                        fq.extend((16 * et, th)
                                  for th in qproj_thunks(et, qTb[et], 512))
                    if h >= 2 and h % 2 == 0:
                        emit_out_group(0, h // 2 - 1)
                        if h == H - 2:
                            emit_out_group(0, NET - 1)
                        fq.extend((9999, th) for th in oq)
                        oq.clear()
                    attention_head(1, h, fq, 1)
                while fq:
                    fq.pop(0)[1]()

                # tail: last head's norm, remaining out-proj groups
                pctx, ph, pqc = pend
                emit_recip(pctx, (pqc, ph))
                emit_bc((pqc, ph))
                emit_bcmul(pctx, (pqc, ph), ph, pqc)
                for et in range(NET):
                    emit_out_group(1, et)
                while oq:
                    oq.pop(0)()
    nc.compile()
    return nc


def _get_nc():
    global _NC_CACHE
    if _NC_CACHE is None:
        _NC_CACHE = build_nc()
    return _NC_CACHE


def _prep_maps(x, Wq, bq, Wk, bk, Wv, bv, Wo):
    bf = ml_dtypes.bfloat16
    WqT = np.ascontiguousarray(Wq.T).astype(bf)
    WkT = np.ascontiguousarray(Wk.T).astype(bf)
    WvT = np.ascontiguousarray(Wv.T).astype(bf)
    WoT = np.ascontiguousarray(Wo.T).astype(bf)
    bqt = np.ascontiguousarray(bq.reshape(NET, 128).T).astype(np.float32)
    bkt = np.ascontiguousarray(bk.reshape(NET, 128).T).astype(np.float32)
    bvr = np.ascontiguousarray(bv.reshape(1, D)).astype(bf)
    in_maps = []
    for c in range(8):
        b, hq = c // 2, c % 2
        xTb = np.ascontiguousarray(x[b].T).astype(bf)  # [D, S]
        if hq == 1:
            # rotate so local query half sits at columns [0, SQ)
            xTb = np.ascontiguousarray(
                np.concatenate([xTb[:, SQ:], xTb[:, :SQ]], axis=1))
        in_maps.append(dict(xT=xTb, WqT=WqT, WkT=WkT, WvT=WvT, WoT=WoT,
                            bqt=bqt, bkt=bkt, bvr=bvr))
    return in_maps


def run(x, Wq, bq, Wk, bk, Wv, bv, Wo, bo, trace=False, **spmd_kwargs):
    nc = _get_nc()
    in_maps = _prep_maps(x, Wq, bq, Wk, bk, Wv, bv, Wo)
    res = run_bass_kernel_spmd(nc, in_maps, core_ids=list(range(8)),
                               trace=trace, **spmd_kwargs)
    out = np.empty((B, S, D), np.float32)
    for c in range(8):
        b, hq = c // 2, c % 2
        out[b, hq * SQ:(hq + 1) * SQ, :] = np.asarray(
            res.results[c]["outT"], np.float32).T
    out += bo.astype(np.float32)
    return out, res


def kernel(x, Wq, bq, Wk, bk, Wv, bv, Wo, bo):
    out, _ = run(np.asarray(x, np.float32), np.asarray(Wq, np.float32),
                 np.asarray(bq, np.float32), np.asarray(Wk, np.float32),
                 np.asarray(bk, np.float32), np.asarray(Wv, np.float32),
                 np.asarray(bv, np.float32), np.asarray(Wo, np.float32),
                 np.asarray(bo, np.float32))
    return out
